# revision 1
# baseline (speedup 1.0000x reference)
"""Multi-head attention (N=4, L=2048, E=1024, H=16) on 8 Trainium2 cores.

Sharding: core c -> (batch n = c // 2, head-group g = c % 2).  Each core
computes, for its batch and its 8 heads (512 embed dims):
  qp_T/kp_T = (W x^T) in [d, tok] layout, vp in [tok, d] layout,
  S_T[k, q] scores with two heads packed in the 128 partitions via PE row
  tiling, exp via ACT with the 1/sqrt(1024) scale folded in, attn@v with a
  ones column appended to vp so the softmax denominator accumulates in the
  same PSUM tile, normalization via a 1-partition PE replicate matmul + DVE
  multiply, then the output projection against Wo columns of this group.
Host sums the two per-group partial outputs per batch and adds bo.

Matmul operands are fp16 (1 cycle/row on the PE at 2.4 GHz, FWL weight
loads); accumulation stays fp32 in PSUM.  fp16 keeps ~5e-4 element
precision, an order better than bf16 at the same speed.
"""

import os

import numpy as np

import concourse.bacc as bacc
import concourse.mybir as mybir
import concourse.tile as tile
from concourse.bass import ds, ts
from concourse.bass_utils import run_bass_kernel_spmd

F32 = mybir.dt.float32
F16 = mybir.dt.float16

E = 1024          # embed
H = 16            # heads (global)
D = 64            # head dim
L = 2048          # sequence length
NB = 4            # batch
GE = 512          # embed dims per head group (8 heads)
P = 128           # partitions
TB = L // 512     # 4 token blocks of 512
QB2 = L // 1024   # 2 q superblocks of 1024
EC = E // P       # 8 embed chunks
DC = GE // P      # 4 d-chunks per group == head pairs
KT = L // P       # 16 key-token chunks

_CACHE = {}


def _build():
    nc = bacc.Bacc("TRN2", debug=False, enable_asserts=False, num_devices=8)

    xq = nc.dram_tensor("xq", [E, L], F16, kind="ExternalInput").ap()
    xk = nc.dram_tensor("xk", [E, L], F16, kind="ExternalInput").ap()
    xv = nc.dram_tensor("xv", [E, L], F16, kind="ExternalInput").ap()
    wq = nc.dram_tensor("wq", [E, GE], F16, kind="ExternalInput").ap()
    wk = nc.dram_tensor("wk", [E, GE], F16, kind="ExternalInput").ap()
    wv = nc.dram_tensor("wv", [E, GE], F16, kind="ExternalInput").ap()
    wo = nc.dram_tensor("wo", [GE, E], F16, kind="ExternalInput").ap()
    bqk = nc.dram_tensor("bqk", [2, P, DC], F32, kind="ExternalInput").ap()
    bvr = nc.dram_tensor("bvr", [1, GE], F16, kind="ExternalInput").ap()
    out = nc.dram_tensor("out", [L, E], F32, kind="ExternalOutput").ap()

    with tile.TileContext(nc) as tc, \
         nc.allow_low_precision(reason="fp16 attention internals by design"):
        with tc.tile_pool(name="persist", bufs=1) as pp, \
             tc.tile_pool(name="wpool", bufs=1) as wp, \
             tc.tile_pool(name="xpool", bufs=3) as xp, \
             tc.tile_pool(name="bias", bufs=1) as bp, \
             tc.tile_pool(name="expp", bufs=4) as ep, \
             tc.tile_pool(name="dtmp", bufs=9) as dt_pool, \
             tc.tile_pool(name="otmp", bufs=3) as ot, \
             tc.tile_pool(name="ppsum", bufs=1, space="PSUM") as pps, \
             tc.tile_pool(name="spsum", bufs=2, space="PSUM") as sps, \
             tc.tile_pool(name="opsum", bufs=1, space="PSUM") as ops, \
             tc.tile_pool(name="rpsum", bufs=1, space="PSUM") as rps:
            # persistent SBUF
            vp = pp.tile([P, KT, 8, D + 1], F16)         # vp_aug per head
            ao = pp.tile([P, DC, L], F16)                # normalized attnout_T
            qs = pp.tile([P, DC, L], F16)                # qp_T  [d, pair, tok]
            ks = pp.tile([P, DC, L], F16)                # kp_T
            ones32 = pp.tile([1, P], F32)
            ones = pp.tile([1, P], F16)
            nc.gpsimd.memset(ones32[:], 1.0)
            nc.vector.tensor_copy(ones[:], ones32[:])

            bq_t = bp.tile([P, DC], F32, tag="bq")
            bk_t = bp.tile([P, DC], F32, tag="bk")
            bv_row = bp.tile([1, GE], F16, tag="bv")
            nc.sync.dma_start(bq_t[:], bqk[0])
            nc.sync.dma_start(bk_t[:], bqk[1])
            nc.sync.dma_start(bv_row[:], bvr)

            wq_sb = wp.tile([P, EC, GE], F16, tag="wq")
            wk_sb = wp.tile([P, EC, GE], F16, tag="wk")
            wv_sb = wp.tile([P, EC, GE], F16, tag="wv")
            wo_sb = wp.tile([P, DC, E], F16, tag="wo")
            nc.sync.dma_start(wq_sb[:], wq.rearrange("(eo p) g -> p eo g", p=P))
            nc.sync.dma_start(wk_sb[:], wk.rearrange("(eo p) g -> p eo g", p=P))
            nc.sync.dma_start(wv_sb[:], wv.rearrange("(eo p) g -> p eo g", p=P))
            nc.sync.dma_start(wo_sb[:], wo.rearrange("(dc p) e -> p dc e", p=P))

            # ---- vp projection: natural [tok, d] layout + ones column ----
            onescol = bp.tile([P, KT], F32, tag="onescol")
            nc.gpsimd.memset(onescol[:], 1.0)
            nc.vector.tensor_copy(
                vp[:, :, :, D : D + 1],
                onescol[:, :, None, None].to_broadcast([P, KT, 8, 1]),
            )
            for tb in range(TB):
                x_sb = xp.tile([P, EC, 512], F16, tag="xslab", name="x_sb")
                nc.sync.dma_start(
                    x_sb[:],
                    xv[:, ts(tb, 512)].rearrange("(eo p) t -> p eo t", p=P),
                )
                for j in range(4):
                    c = tb * 4 + j
                    ps_t = pps.tile([P, GE], F32, tag="pp")
                    for e in range(EC):
                        nc.tensor.matmul(
                            ps_t[:],
                            x_sb[:, e, ts(j, P)],
                            wv_sb[:, e, :],
                            start=(e == 0),
                            stop=False,
                        )
                    nc.tensor.matmul(
                        ps_t[:], ones[:, :P], bv_row[:], start=False, stop=True
                    )
                    nc.vector.tensor_copy(
                        vp[:, c, :, 0:D],
                        ps_t.rearrange("p (h d) -> p h d", d=D),
                    )

            # ---- per head-pair: q/k projections then attention ----
            pending = []
            for pr in range(DC):
                for x_ap, w_sb, b_t, st in [
                    (xq, wq_sb, bq_t, qs),
                    (xk, wk_sb, bk_t, ks),
                ]:
                    for tb in range(TB):
                        x_sb = xp.tile([P, EC, 512], F16, tag="xslab", name="x_sb")
                        nc.sync.dma_start(
                            x_sb[:],
                            x_ap[:, ts(tb, 512)].rearrange(
                                "(eo p) t -> p eo t", p=P
                            ),
                        )
                        ps_t = pps.tile([P, 512], F32, tag="pp")
                        for e in range(EC):
                            nc.tensor.matmul(
                                ps_t[:],
                                w_sb[:, e, ts(pr, P)],
                                x_sb[:, e, :],
                                start=(e == 0),
                                stop=(e == EC - 1),
                            )
                        nc.vector.tensor_scalar_add(
                            st[:, pr, ts(tb, 512)], ps_t[:], b_t[:, pr : pr + 1]
                        )

                for qb in range(TB):
                    # deferred normalization of the previous block: by now its
                    # reciprocal has finished, so the replicate matmul does not
                    # stall the in-order PE queue
                    while len(pending) > 4:
                        i_, pr_, qb_, sb_o_, dinv_ = pending.pop(0)
                        ps_r = rps.tile([P, 512], F32, tag="rf", name="ps_r")
                        nc.tensor.matmul(
                            ps_r[0:D, :], ones[:, :D], dinv_[:],
                            start=True, stop=True,
                        )
                        rep_sb = dt_pool.tile([D, 512], F32, tag="repsb")
                        nc.vector.tensor_copy(rep_sb[:], ps_r[0:D, :])
                        nc.vector.tensor_tensor(
                            ao[ds(D * i_, D), pr_, ts(qb_, 512)],
                            sb_o_[0:D, :],
                            rep_sb[:],
                            mybir.AluOpType.mult,
                        )
                    ps_oo = [
                        ops.tile([P, 512], F32, tag=f"ov{i}", name=f"ov{i}")
                        for i in range(2)
                    ]
                    for kt in range(KT):
                        ps_s = sps.tile([P, 1024], F32, tag="sc")
                        for i in range(2):
                            nc.tensor.matmul(
                                ps_s[:, ts(i, 512)],
                                ks[ds(64 * i, 64), pr, ts(kt, P)],
                                qs[ds(64 * i, 64), pr, ts(qb, 512)],
                                start=True,
                                stop=True,
                                tile_position=(64 * i, 0),
                            )
                        e_t = ep.tile([P, 1024], F16, tag="exp", name="e_t")
                        nc.scalar.activation(
                            e_t[:],
                            ps_s[:],
                            mybir.ActivationFunctionType.Exp,
                            scale=float(1.0 / 32.0),
                        )
                        for i in range(2):
                            nc.tensor.matmul(
                                ps_oo[i][0 : D + 1, :],
                                vp[:, kt, 2 * pr + i, :],
                                e_t[:, ts(i, 512)],
                                start=(kt == 0),
                                stop=(kt == KT - 1),
                            )
                    for i in range(2):
                        ps_o = ps_oo[i]
                        # one fast copy releases the PSUM bank; reciprocal runs
                        # on DVE while the NEXT block's attention proceeds
                        sb_o = dt_pool.tile([D + 1, 512], F32, tag="sbo", name="sb_o")
                        nc.vector.tensor_copy(sb_o[:], ps_o[0 : D + 1, :])
                        # 1/denom via exp(-ln(x)) on ACT: keeps the slow DVE
                        # reciprocal out of the DVE queue, whose ticks gate
                        # PE instructions downstream
                        lnv = dt_pool.tile([1, 512], F32, tag="lnv")
                        nc.scalar.activation(
                            lnv[:], sb_o[D : D + 1, :],
                            mybir.ActivationFunctionType.Ln,
                        )
                        dinv = dt_pool.tile([1, 512], F16, tag="dinv")
                        nc.scalar.activation(
                            dinv[:], lnv[:],
                            mybir.ActivationFunctionType.Exp,
                            scale=-1.0,
                        )
                        pending.append((i, pr, qb, sb_o, dinv))

            # flush the last block's deferred normalization
            for (i_, pr_, qb_, sb_o_, dinv_) in pending:
                ps_r = rps.tile([P, 512], F32, tag="rf", name="ps_r")
                nc.tensor.matmul(
                    ps_r[0:D, :], ones[:, :D], dinv_[:], start=True, stop=True
                )
                rep_sb = dt_pool.tile([D, 512], F32, tag="repsb")
                nc.vector.tensor_copy(rep_sb[:], ps_r[0:D, :])
                nc.vector.tensor_tensor(
                    ao[ds(D * i_, D), pr_, ts(qb_, 512)],
                    sb_o_[0:D, :],
                    rep_sb[:],
                    mybir.AluOpType.mult,
                )
            pending = []

            # ---- output projection ----
            for tb in range(KT):
                for ob in range(2):
                    ps_f = rps.tile([P, 512], F32, tag="rf", name="ps_f")
                    for dc in range(DC):
                        nc.tensor.matmul(
                            ps_f[:],
                            ao[:, dc, ts(tb, P)],
                            wo_sb[:, dc, ts(ob, 512)],
                            start=(dc == 0),
                            stop=(dc == DC - 1),
                        )
                    o_t = ot.tile([P, 512], F32, tag="fout")
                    nc.vector.tensor_copy(o_t[:], ps_f[:])
                    nc.sync.dma_start(out[ts(tb, P), ts(ob, 512)], o_t[:])

    nc.compile()
    return nc


def kernel(q, k, v, padding_mask, sequence_mask, Wq, bq, Wk, bk, Wv, bv, Wo, bo):
    # masks intentionally unused: the reference discards masked_fill results.
    if "nc" not in _CACHE:
        _CACHE["nc"] = _build()
    nc = _CACHE["nc"]

    q = np.asarray(q, np.float32)
    k = np.asarray(k, np.float32)
    v = np.asarray(v, np.float32)
    Wq = np.asarray(Wq, np.float32)
    Wk = np.asarray(Wk, np.float32)
    Wv = np.asarray(Wv, np.float32)
    Wo = np.asarray(Wo, np.float32)
    bq = np.asarray(bq, np.float32)
    bk = np.asarray(bk, np.float32)
    bv = np.asarray(bv, np.float32)
    bo = np.asarray(bo, np.float32)

    in_maps = []
    for c in range(8):
        n, g = c // 2, c % 2
        sl = slice(g * GE, (g + 1) * GE)
        bqk_arr = np.stack(
            [
                bq[sl].reshape(DC, P).T,
                bk[sl].reshape(DC, P).T,
            ]
        ).astype(np.float32)
        in_maps.append(
            {
                "xq": np.ascontiguousarray(q[n].T.astype(np.float16)),
                "xk": np.ascontiguousarray(k[n].T.astype(np.float16)),
                "xv": np.ascontiguousarray(v[n].T.astype(np.float16)),
                "wq": np.ascontiguousarray(Wq[sl, :].T.astype(np.float16)),
                "wk": np.ascontiguousarray(Wk[sl, :].T.astype(np.float16)),
                "wv": np.ascontiguousarray(Wv[sl, :].T.astype(np.float16)),
                "wo": np.ascontiguousarray(Wo[:, sl].T.astype(np.float16)),
                "bqk": np.ascontiguousarray(bqk_arr),
                "bvr": np.ascontiguousarray(bv[sl][None, :].astype(np.float16)),
            }
        )

    trace = os.environ.get("KERNEL_TRACE") == "1"
    kw = {}
    if trace:
        kw = dict(trace=True, trace_cores=list(range(8)))
    res = run_bass_kernel_spmd(nc, in_maps, core_ids=list(range(8)), **kw)
    if trace:
        _CACHE["exec_time_ns"] = res.exec_time_ns
        _CACHE["mean_exec_time_ns"] = res.mean_exec_time_ns

    outp = np.empty((NB, L, E), np.float32)
    for n in range(NB):
        outp[n] = (
            res.results[2 * n]["out"] + res.results[2 * n + 1]["out"] + bo[None, :]
        )
    return outp



# revision 5
# speedup vs baseline: 1.4478x; 1.4478x over previous
"""Multi-head attention (N=4, L=2048, E=1024, H=16) on 8 Trainium2 cores.

Sharding: core c -> (batch n = c // 2, head-group g = c % 2).  Each core
computes, for its batch and its 8 heads (512 embed dims):
  qp_T/kp_T = (W x^T) in [d, tok] layout, vp in [tok, d] layout,
  S_T[k, q] scores with two heads packed in the 128 partitions via PE row
  tiling, exp via ACT with the 1/sqrt(1024) scale folded in, attn@v with a
  ones column appended to vp so the softmax denominator accumulates in the
  same PSUM tile, normalization via DVE reciprocal_approx_fast + a col-tiled
  PE replicate matmul + one DVE multiply, then the output projection.
Host sums the two per-group partial outputs per batch and adds bo.

Pipeline structure: the softmax exp stream on the ACT engine is the
critical path (~256 x ~1.1us).  All other work (v projection, next pair's
q/k projections, the output projection, normalization) is interleaved into
the attention blocks as PE/DVE filler so ACT never waits:
  pair 0 block (0,0): v-projection chunks emitted just ahead of their attnv
  pair p blocks:      next pair's q/k projection chunks as fillers
  pair 3 blocks:      output projection pieces for finished q-blocks
The softmax denominator reciprocal runs on DVE (reciprocal_approx_fast),
keeping LN/EXP table reloads off the ACT engine entirely.

Matmul operands are fp16 (1 col/cycle on the PE at 2.4 GHz, FWL weight
loads); accumulation stays fp32 in PSUM.
"""

import os

import numpy as np

import concourse.bacc as bacc
import concourse.mybir as mybir
import concourse.tile as tile
from concourse.bass import ds, ts
from concourse.bass_utils import run_bass_kernel_spmd

F32 = mybir.dt.float32
F16 = mybir.dt.float16

E = 1024          # embed
H = 16            # heads (global)
D = 64            # head dim
L = 2048          # sequence length
NB = 4            # batch
GE = 512          # embed dims per head group (8 heads)
P = 128           # partitions
TB = L // 512     # 4 token blocks of 512
EC = E // P       # 8 embed chunks
DC = GE // P      # 4 d-chunks per group == head pairs
KT = L // P       # 16 key-token chunks
LAG = 2           # attnv trails scores by LAG kt-chunks

_CACHE = {}


def _build():
    nc = bacc.Bacc("TRN2", debug=False, enable_asserts=False, num_devices=8)

    xq = nc.dram_tensor("xq", [E, L], F16, kind="ExternalInput").ap()
    xk = nc.dram_tensor("xk", [E, L], F16, kind="ExternalInput").ap()
    xv = nc.dram_tensor("xv", [E, L], F16, kind="ExternalInput").ap()
    wq = nc.dram_tensor("wq", [E, GE], F16, kind="ExternalInput").ap()
    wk = nc.dram_tensor("wk", [E, GE], F16, kind="ExternalInput").ap()
    wv = nc.dram_tensor("wv", [E, GE], F16, kind="ExternalInput").ap()
    wo = nc.dram_tensor("wo", [GE, E], F16, kind="ExternalInput").ap()
    bqk = nc.dram_tensor("bqk", [2, P, DC], F32, kind="ExternalInput").ap()
    bvr = nc.dram_tensor("bvr", [1, GE], F16, kind="ExternalInput").ap()
    out = nc.dram_tensor("out", [L, E], F32, kind="ExternalOutput").ap()

    with tile.TileContext(nc) as tc, \
         nc.allow_low_precision(reason="fp16 attention internals by design"):
        with tc.tile_pool(name="persist", bufs=1) as pp, \
             tc.tile_pool(name="wpool", bufs=1) as wp, \
             tc.tile_pool(name="xpool", bufs=4) as xp, \
             tc.tile_pool(name="bias", bufs=1) as bp, \
             tc.tile_pool(name="expp", bufs=4) as ep, \
             tc.tile_pool(name="nrm", bufs=2) as npool, \
             tc.tile_pool(name="otmp", bufs=3) as ot, \
             tc.tile_pool(name="ppsum", bufs=2, space="PSUM") as pps, \
             tc.tile_pool(name="spsum", bufs=2, space="PSUM") as sps, \
             tc.tile_pool(name="opsum", bufs=1, space="PSUM") as ops:
            # persistent SBUF
            vp = pp.tile([P, KT, 8, D + 1], F16)         # vp_aug per head
            ao = pp.tile([P, DC, L], F16)                # normalized attnout_T
            qs = pp.tile([P, DC, L], F16)                # qp_T  [d, pair, tok]
            ks = pp.tile([P, DC, L], F16)                # kp_T
            ones32 = pp.tile([1, P], F32)
            ones = pp.tile([1, P], F16)
            nc.gpsimd.memset(ones32[:], 1.0)
            nc.vector.tensor_copy(ones[:], ones32[:])

            bq_t = bp.tile([P, DC], F32, tag="bq")
            bk_t = bp.tile([P, DC], F32, tag="bk")
            bv_row = bp.tile([1, GE], F16, tag="bv")
            nc.sync.dma_start(bq_t[:], bqk[0])
            nc.sync.dma_start(bk_t[:], bqk[1])
            nc.sync.dma_start(bv_row[:], bvr)

            wq_sb = wp.tile([P, EC, GE], F16, tag="wq")
            wk_sb = wp.tile([P, EC, GE], F16, tag="wk")
            wv_sb = wp.tile([P, EC, GE], F16, tag="wv")
            wo_sb = wp.tile([P, DC, E], F16, tag="wo")
            nc.sync.dma_start(wk_sb[:], wk.rearrange("(eo p) g -> p eo g", p=P))
            nc.sync.dma_start(wq_sb[:], wq.rearrange("(eo p) g -> p eo g", p=P))
            nc.sync.dma_start(wv_sb[:], wv.rearrange("(eo p) g -> p eo g", p=P))
            nc.sync.dma_start(wo_sb[:], wo.rearrange("(dc p) e -> p dc e", p=P))

            # ones column of vp_aug (softmax denominator accumulator)
            onescol = bp.tile([P, KT], F32, tag="onescol")
            nc.gpsimd.memset(onescol[:], 1.0)
            nc.vector.tensor_copy(
                vp[:, :, :, D : D + 1],
                onescol[:, :, None, None].to_broadcast([P, KT, 8, 1]),
            )

            # ---------------- emission helpers ----------------
            def load_slab(x_ap, tb):
                x_sb = xp.tile([P, EC, 512], F16, tag="xslab", name="x_sb")
                nc.sync.dma_start(
                    x_sb[:],
                    x_ap[:, ts(tb, 512)].rearrange("(eo p) t -> p eo t", p=P),
                )
                return x_sb

            def qk_chunk(pr, st, w_sb, b_t, tb, slab):
                ps_t = pps.tile([P, 512], F32, tag="pp")
                for e in range(EC):
                    nc.tensor.matmul(
                        ps_t[:],
                        w_sb[:, e, ts(pr, P)],
                        slab[:, e, :],
                        start=(e == 0),
                        stop=(e == EC - 1),
                    )
                nc.vector.tensor_scalar_add(
                    st[:, pr, ts(tb, 512)], ps_t[:], b_t[:, pr : pr + 1]
                )

            def vproj_chunk(c, slab):
                ps_t = pps.tile([P, 512], F32, tag="pp")
                for e in range(EC):
                    nc.tensor.matmul(
                        ps_t[:],
                        slab[:, e, ts(c % 4, P)],
                        wv_sb[:, e, :],
                        start=(e == 0),
                        stop=False,
                    )
                nc.tensor.matmul(
                    ps_t[:], ones[:, :P], bv_row[:], start=False, stop=True
                )
                nc.vector.tensor_copy(
                    vp[:, c, :, 0:D],
                    ps_t.rearrange("p (h d) -> p h d", d=D),
                )

            def oproj_piece(tb, ob):
                ps_f = pps.tile([P, 512], F32, tag="pp")
                for dc in range(DC):
                    nc.tensor.matmul(
                        ps_f[:],
                        ao[:, dc, ts(tb, P)],
                        wo_sb[:, dc, ts(ob, 512)],
                        start=(dc == 0),
                        stop=(dc == DC - 1),
                    )
                o_t = ot.tile([P, 512], F32, tag="fout")
                nc.vector.tensor_copy(o_t[:], ps_f[:])
                nc.sync.dma_start(out[ts(tb, P), ts(ob, 512)], o_t[:])

            # normalization part 1 (block end): drain attnv PSUM into SBUF
            def norm_gather(ps_oo):
                sbp = npool.tile([P, 512], F32, tag="sbp", name="sbp")
                dn = npool.tile([1, 2, 512], F32, tag="dn", name="dn")
                nc.vector.tensor_copy(sbp[0:D, :], ps_oo[0][0:D, :])
                nc.vector.tensor_copy(sbp[D:P, :], ps_oo[1][0:D, :])
                nc.vector.tensor_copy(dn[:, 0, :], ps_oo[0][D : D + 1, :])
                nc.vector.tensor_copy(dn[:, 1, :], ps_oo[1][D : D + 1, :])
                return sbp, dn

            # normalization part 2 (mid next block): reciprocal + replicate
            def norm_finish(pr, qb, sbp, dn):
                dninv = npool.tile([1, 2, 512], F32, tag="dninv", name="dninv")
                nc.vector.reciprocal_approx_fast(dninv[:], dn[:])
                dinv = npool.tile([1, 2, 512], F16, tag="dinv", name="dinv")
                nc.vector.tensor_copy(dinv[:], dninv[:])
                ps_r = pps.tile([P, 512], F32, tag="pp", name="ps_r")
                nc.tensor.matmul(
                    ps_r[0:D, :], ones[:, :D], dinv[:, 0, :],
                    start=True, stop=True,
                )
                nc.tensor.matmul(
                    ps_r[D:P, :], ones[:, :D], dinv[:, 1, :],
                    start=True, stop=True, tile_position=(0, 64),
                )
                nc.vector.tensor_tensor(
                    ao[:, pr, ts(qb, 512)],
                    sbp[:],
                    ps_r[:],
                    mybir.AluOpType.mult,
                )

            # ---------------- q/k projection of pair 0 (startup) ----------
            for x_ap, w_sb, b_t, st in [
                (xk, wk_sb, bk_t, ks),
                (xq, wq_sb, bq_t, qs),
            ]:
                for tb in range(TB):
                    slab = load_slab(x_ap, tb)
                    qk_chunk(0, st, w_sb, b_t, tb, slab)

            # ---------------- attention with interleaved fillers ----------
            pending_norm = None     # (pr, qb, sbp, dn)
            vslabs = {}

            for pr in range(DC):
                # filler tasks for this pair's 4 blocks: list of callables
                fillers = [[] for _ in range(TB)]
                if pr < DC - 1:
                    # next pair's q/k projection: 8 chunks over blocks 1..3
                    npr = pr + 1
                    chunks = []
                    for x_ap, w_sb, b_t, st in [
                        (xk, wk_sb, bk_t, ks),
                        (xq, wq_sb, bq_t, qs),
                    ]:
                        for tb in range(TB):
                            chunks.append((x_ap, w_sb, b_t, st, tb))

                    def mk_qk(npr, x_ap, w_sb, b_t, st, tb):
                        holder = {}

                        def prefetch():
                            holder["slab"] = load_slab(x_ap, tb)

                        def run():
                            qk_chunk(npr, st, w_sb, b_t, tb, holder["slab"])

                        return prefetch, run

                    start_qb = 0 if pr > 0 else 1
                    tasks = []
                    for ch in chunks:
                        pf, run = mk_qk(npr, *ch)
                        tasks.append((pf, run))
                    # distribute over available blocks; prefetch one task ahead
                    nblk = TB - start_qb
                    for i, (pf, run) in enumerate(tasks):
                        blk = start_qb + min(i * nblk // len(tasks), nblk - 1)
                        fillers[blk].append((pf, run))
                else:
                    # pair 3: output projection for finished q-blocks
                    def mk_op(tb, ob):
                        return (None, lambda: oproj_piece(tb, ob))

                    for j in range(TB - 1):      # oproj for qb j in block j+1
                        for tb in range(4 * j, 4 * j + 4):
                            for ob in range(2):
                                fillers[j + 1].append(mk_op(tb, ob))

                for qb in range(TB):
                    # emit prefetches for this block's fillers up front
                    for pf, _ in fillers[qb]:
                        if pf is not None:
                            pf()
                    fq = [run for _, run in fillers[qb]]
                    fi = 0

                    ps_oo = [
                        ops.tile([P, 512], F32, tag=f"ov{i}", name=f"ov{i}")
                        for i in range(2)
                    ]
                    ets = {}
                    first_blk = pr == 0 and qb == 0
                    for kt in range(KT):
                        # scores for both heads of the pair (row-tiled pair,
                        # runs concurrently on the PE)
                        ps_s = sps.tile([P, 1024], F32, tag="sc")
                        for i in range(2):
                            nc.tensor.matmul(
                                ps_s[:, ts(i, 512)],
                                ks[ds(64 * i, 64), pr, ts(kt, P)],
                                qs[ds(64 * i, 64), pr, ts(qb, 512)],
                                start=True,
                                stop=True,
                                tile_position=(64 * i, 0),
                            )
                        e_t = ep.tile([P, 1024], F16, tag="exp", name="e_t")
                        nc.scalar.activation(
                            e_t[:],
                            ps_s[:],
                            mybir.ActivationFunctionType.Exp,
                            scale=float(1.0 / 32.0),
                        )
                        ets[kt] = e_t

                        if first_blk:
                            # v projection races ahead of attnv consumption
                            if kt % 4 == 0:
                                vslabs[kt // 4] = load_slab(xv, kt // 4)
                            vproj_chunk(kt, vslabs[kt // 4])
                        # pair 3's oproj fillers read ao written by
                        # norm_finish, so it must be emitted before them
                        nf_kt = 1 if pr == DC - 1 else 5
                        if kt == nf_kt and pending_norm is not None:
                            norm_finish(*pending_norm)
                            pending_norm = None
                        if (
                            kt % 2 == 1
                            and (pr != DC - 1 or kt >= 3)
                            and fi < len(fq)
                        ):
                            fq[fi]()
                            fi += 1
                        if kt >= LAG:
                            k2 = kt - LAG
                            for i in range(2):
                                nc.tensor.matmul(
                                    ps_oo[i][0 : D + 1, :],
                                    vp[:, k2, 2 * pr + i, :],
                                    ets[k2][:, ts(i, 512)],
                                    start=(k2 == 0),
                                    stop=False,
                                )
                    for k2 in range(KT - LAG, KT):
                        for i in range(2):
                            nc.tensor.matmul(
                                ps_oo[i][0 : D + 1, :],
                                vp[:, k2, 2 * pr + i, :],
                                ets[k2][:, ts(i, 512)],
                                start=False,
                                stop=(k2 == KT - 1),
                            )
                    while fi < len(fq):
                        fq[fi]()
                        fi += 1
                    sbp, dn = norm_gather(ps_oo)
                    pending_norm = (pr, qb, sbp, dn)

            # tail: last block's normalization + its output projection
            norm_finish(*pending_norm)
            pending_norm = None
            for tb in range(4 * (TB - 1), 4 * TB):
                for ob in range(2):
                    oproj_piece(tb, ob)

    nc.compile()
    return nc


def kernel(q, k, v, padding_mask, sequence_mask, Wq, bq, Wk, bk, Wv, bv, Wo, bo):
    # masks intentionally unused: the reference discards masked_fill results.
    if "nc" not in _CACHE:
        _CACHE["nc"] = _build()
    nc = _CACHE["nc"]

    q = np.asarray(q, np.float32)
    k = np.asarray(k, np.float32)
    v = np.asarray(v, np.float32)
    Wq = np.asarray(Wq, np.float32)
    Wk = np.asarray(Wk, np.float32)
    Wv = np.asarray(Wv, np.float32)
    Wo = np.asarray(Wo, np.float32)
    bq = np.asarray(bq, np.float32)
    bk = np.asarray(bk, np.float32)
    bv = np.asarray(bv, np.float32)
    bo = np.asarray(bo, np.float32)

    in_maps = []
    for c in range(8):
        n, g = c // 2, c % 2
        sl = slice(g * GE, (g + 1) * GE)
        bqk_arr = np.stack(
            [
                bq[sl].reshape(DC, P).T,
                bk[sl].reshape(DC, P).T,
            ]
        ).astype(np.float32)
        in_maps.append(
            {
                "xq": np.ascontiguousarray(q[n].T.astype(np.float16)),
                "xk": np.ascontiguousarray(k[n].T.astype(np.float16)),
                "xv": np.ascontiguousarray(v[n].T.astype(np.float16)),
                "wq": np.ascontiguousarray(Wq[sl, :].T.astype(np.float16)),
                "wk": np.ascontiguousarray(Wk[sl, :].T.astype(np.float16)),
                "wv": np.ascontiguousarray(Wv[sl, :].T.astype(np.float16)),
                "wo": np.ascontiguousarray(Wo[:, sl].T.astype(np.float16)),
                "bqk": np.ascontiguousarray(bqk_arr),
                "bvr": np.ascontiguousarray(bv[sl][None, :].astype(np.float16)),
            }
        )

    trace = os.environ.get("KERNEL_TRACE") == "1"
    kw = {}
    if trace:
        kw = dict(trace=True, trace_cores=list(range(8)))
    res = run_bass_kernel_spmd(nc, in_maps, core_ids=list(range(8)), **kw)
    if trace:
        _CACHE["exec_time_ns"] = res.exec_time_ns
        _CACHE["mean_exec_time_ns"] = res.mean_exec_time_ns

    outp = np.empty((NB, L, E), np.float32)
    for n in range(NB):
        outp[n] = (
            res.results[2 * n]["out"] + res.results[2 * n + 1]["out"] + bo[None, :]
        )
    return outp


# revision 6
# speedup vs baseline: 1.5062x; 1.0403x over previous
"""Multi-head attention (N=4, L=2048, E=1024, H=16) on 8 Trainium2 cores.

Sharding: core c -> (batch n = c // 2, head-group g = c % 2).  Each core
computes, for its batch and its 8 heads (512 embed dims):
  qp_T/kp_T = (W x^T) in [d, tok] layout (fp8 DoubleRow matmuls; the x64
  fp8 weight scale folds into the softmax exp scale), vp in [tok, d]
  layout (fp16), S_T[k, q] scores with two heads packed in the 128
  partitions via PE row tiling, exp via ACT, attn@v with a ones column
  appended to vp so the softmax denominator accumulates in the same PSUM
  tile, normalization via DVE reciprocal_approx_fast + a col-tiled PE
  replicate matmul + one DVE multiply, then the output projection (fp16).
Host sums the two per-group partial outputs per batch and adds bo.

Pipeline structure: the softmax exp stream on the ACT engine is the
critical path (~256 x ~1.05us).  All other work (v projection, next
pair's q/k projections, the output projection, normalization) is
interleaved into the attention blocks as PE/DVE filler so ACT never
waits:
  pair 0 block (0,0): v-projection chunks emitted just ahead of attnv
  pair p blocks:      next pair's q/k projection chunks as fillers
  pair 3 blocks:      output projection pieces for finished q-blocks
"""

import os

import numpy as np

import concourse.bacc as bacc
import concourse.mybir as mybir
import concourse.tile as tile
from concourse.bass import ds, ts
from concourse.bass_utils import run_bass_kernel_spmd

F32 = mybir.dt.float32
F16 = mybir.dt.float16
F8 = mybir.dt.float8e4

E = 1024          # embed
H = 16            # heads (global)
D = 64            # head dim
L = 2048          # sequence length
NB = 4            # batch
GE = 512          # embed dims per head group (8 heads)
P = 128           # partitions
TB = L // 512     # 4 token blocks of 512
EC = E // P       # 8 embed chunks
DC = GE // P      # 4 d-chunks per group == head pairs
KT = L // P       # 16 key-token chunks
WS = 64.0         # fp8 weight scale for q/k projections

_CACHE = {}


def _build():
    nc = bacc.Bacc("TRN2", debug=False, enable_asserts=False, num_devices=8)

    xq = nc.dram_tensor("xq", [E, L], F8, kind="ExternalInput").ap()
    xk = nc.dram_tensor("xk", [E, L], F8, kind="ExternalInput").ap()
    xv = nc.dram_tensor("xv", [E, L], F16, kind="ExternalInput").ap()
    wq = nc.dram_tensor("wq", [E, GE], F8, kind="ExternalInput").ap()
    wk = nc.dram_tensor("wk", [E, GE], F8, kind="ExternalInput").ap()
    wv = nc.dram_tensor("wv", [E, GE], F16, kind="ExternalInput").ap()
    wo = nc.dram_tensor("wo", [GE, E], F16, kind="ExternalInput").ap()
    bqk = nc.dram_tensor("bqk", [2, P, DC], F32, kind="ExternalInput").ap()
    bvr = nc.dram_tensor("bvr", [1, GE], F16, kind="ExternalInput").ap()
    out = nc.dram_tensor("out", [L, E], F32, kind="ExternalOutput").ap()

    with tile.TileContext(nc) as tc, \
         nc.allow_low_precision(reason="fp16/fp8 attention internals by design"):
        with tc.tile_pool(name="persist", bufs=1) as pp, \
             tc.tile_pool(name="wpool", bufs=1) as wp, \
             tc.tile_pool(name="xpool", bufs=8) as xp, \
             tc.tile_pool(name="vxpool", bufs=4) as vxp, \
             tc.tile_pool(name="bias", bufs=1) as bp, \
             tc.tile_pool(name="expp", bufs=6) as ep, \
             tc.tile_pool(name="nrm", bufs=2) as npool, \
             tc.tile_pool(name="otmp", bufs=3) as ot, \
             tc.tile_pool(name="ppsum", bufs=2, space="PSUM") as pps, \
             tc.tile_pool(name="spsum", bufs=2, space="PSUM") as sps, \
             tc.tile_pool(name="opsum", bufs=1, space="PSUM") as ops:
            # persistent SBUF
            vp = pp.tile([P, KT, 8, D + 1], F16)         # vp_aug per head
            ao = pp.tile([P, DC, L], F16)                # normalized attnout_T
            qs = pp.tile([P, DC, L], F16)                # qp_T  [d, pair, tok]
            ks = pp.tile([P, DC, L], F16)                # kp_T
            ones32 = pp.tile([1, P], F32)
            ones = pp.tile([1, P], F16)
            nc.gpsimd.memset(ones32[:], 1.0)
            nc.vector.tensor_copy(ones[:], ones32[:])

            bq_t = bp.tile([P, DC], F32, tag="bq")
            bk_t = bp.tile([P, DC], F32, tag="bk")
            bv_row = bp.tile([1, GE], F16, tag="bv")
            nc.sync.dma_start(bq_t[:], bqk[0])
            nc.sync.dma_start(bk_t[:], bqk[1])
            nc.sync.dma_start(bv_row[:], bvr)

            wq_sb = wp.tile([P, EC, GE], F8, tag="wq")
            wk_sb = wp.tile([P, EC, GE], F8, tag="wk")
            wv_sb = wp.tile([P, EC, GE], F16, tag="wv")
            wo_sb = wp.tile([P, DC, E], F16, tag="wo")
            nc.sync.dma_start(wk_sb[:], wk.rearrange("(eo p) g -> p eo g", p=P))
            nc.sync.dma_start(wq_sb[:], wq.rearrange("(eo p) g -> p eo g", p=P))
            nc.sync.dma_start(wv_sb[:], wv.rearrange("(eo p) g -> p eo g", p=P))
            nc.sync.dma_start(wo_sb[:], wo.rearrange("(dc p) e -> p dc e", p=P))

            # ones column of vp_aug (softmax denominator accumulator)
            onescol = bp.tile([P, KT], F32, tag="onescol")
            nc.gpsimd.memset(onescol[:], 1.0)
            nc.vector.tensor_copy(
                vp[:, :, :, D : D + 1],
                onescol[:, :, None, None].to_broadcast([P, KT, 8, 1]),
            )

            # ---------------- emission helpers ----------------
            def load_slab(x_ap, tb):
                x_sb = xp.tile([P, EC, 512], F8, tag="xslab", name="x_sb")
                nc.sync.dma_start(
                    x_sb[:],
                    x_ap[:, ts(tb, 512)].rearrange("(eo p) t -> p eo t", p=P),
                )
                return x_sb

            def load_vslab(tb):
                x_sb = vxp.tile([P, EC, 512], F16, tag="vslab", name="v_sb")
                nc.sync.dma_start(
                    x_sb[:],
                    xv[:, ts(tb, 512)].rearrange("(eo p) t -> p eo t", p=P),
                )
                return x_sb

            def qk_chunk(pr, st, w_sb, b_t, tb, slab):
                # fp8 DoubleRow: 2 contraction rows per PE cell
                ps_t = pps.tile([P, 512], F32, tag="pp")
                for e2 in range(EC // 2):
                    nc.tensor.matmul(
                        ps_t[:],
                        w_sb[:, 2 * e2 : 2 * e2 + 2, ts(pr, P)],
                        slab[:, 2 * e2 : 2 * e2 + 2, :],
                        start=(e2 == 0),
                        stop=(e2 == EC // 2 - 1),
                        perf_mode=mybir.MatmulPerfMode.DoubleRow,
                    )
                nc.vector.tensor_scalar_add(
                    st[:, pr, ts(tb, 512)], ps_t[:], b_t[:, pr : pr + 1]
                )

            def vproj_chunk(c, slab):
                ps_t = pps.tile([P, 512], F32, tag="pp")
                for e in range(EC):
                    nc.tensor.matmul(
                        ps_t[:],
                        slab[:, e, ts(c % 4, P)],
                        wv_sb[:, e, :],
                        start=(e == 0),
                        stop=False,
                    )
                nc.tensor.matmul(
                    ps_t[:], ones[:, :P], bv_row[:], start=False, stop=True
                )
                nc.vector.tensor_copy(
                    vp[:, c, :, 0:D],
                    ps_t.rearrange("p (h d) -> p h d", d=D),
                )

            def oproj_piece(tb, ob):
                ps_f = pps.tile([P, 512], F32, tag="pp")
                for dc in range(DC):
                    nc.tensor.matmul(
                        ps_f[:],
                        ao[:, dc, ts(tb, P)],
                        wo_sb[:, dc, ts(ob, 512)],
                        start=(dc == 0),
                        stop=(dc == DC - 1),
                    )
                o_t = ot.tile([P, 512], F32, tag="fout")
                nc.vector.tensor_copy(o_t[:], ps_f[:])
                nc.sync.dma_start(out[ts(tb, P), ts(ob, 512)], o_t[:])

            # normalization part 1 (block end): drain attnv PSUM into SBUF
            def norm_gather(ps_oo):
                sbp = npool.tile([P, 512], F32, tag="sbp", name="sbp")
                dn = npool.tile([1, 2, 512], F32, tag="dn", name="dn")
                nc.vector.tensor_copy(sbp[0:D, :], ps_oo[0][0:D, :])
                nc.vector.tensor_copy(sbp[D:P, :], ps_oo[1][0:D, :])
                nc.vector.tensor_copy(dn[:, 0, :], ps_oo[0][D : D + 1, :])
                nc.vector.tensor_copy(dn[:, 1, :], ps_oo[1][D : D + 1, :])
                return sbp, dn

            # normalization part 2 (mid next block): reciprocal + replicate
            def norm_finish(pr, qb, sbp, dn):
                dninv = npool.tile([1, 2, 512], F32, tag="dninv", name="dninv")
                nc.vector.reciprocal_approx_fast(dninv[:], dn[:])
                dinv = npool.tile([1, 2, 512], F16, tag="dinv", name="dinv")
                nc.vector.tensor_copy(dinv[:], dninv[:])
                ps_r = pps.tile([P, 512], F32, tag="pp", name="ps_r")
                nc.tensor.matmul(
                    ps_r[0:D, :], ones[:, :D], dinv[:, 0, :],
                    start=True, stop=True,
                )
                nc.tensor.matmul(
                    ps_r[D:P, :], ones[:, :D], dinv[:, 1, :],
                    start=True, stop=True, tile_position=(0, 64),
                )
                nc.vector.tensor_tensor(
                    ao[:, pr, ts(qb, 512)],
                    sbp[:],
                    ps_r[:],
                    mybir.AluOpType.mult,
                )

            # ---------------- startup: pair-0 q/k projection --------------
            # kproj feeds the scores' stationary operand (all 2048 tokens);
            # qproj tb=0 feeds q-block 0.  xv slabs prefetch right behind so
            # the v projection can start early in block (0,0).
            kslabs = [load_slab(xk, tb) for tb in range(TB)]
            qslab0 = load_slab(xq, 0)
            vslabs = {tb: load_vslab(tb) for tb in range(TB)}
            for tb in range(TB):
                qk_chunk(0, ks, wk_sb, bk_t, tb, kslabs[tb])
            qk_chunk(0, qs, wq_sb, bq_t, 0, qslab0)
            for tb in range(1, TB):
                qk_chunk(0, qs, wq_sb, bq_t, tb, load_slab(xq, tb))

            # ---------------- attention with interleaved fillers ----------
            pending_norm = None     # (pr, qb, sbp, dn)

            for pr in range(DC):
                # filler tasks for this pair's 4 blocks: list of callables
                fillers = [[] for _ in range(TB)]
                if pr < DC - 1:
                    # next pair's q/k projection: 8 chunks
                    npr = pr + 1
                    chunks = []
                    for x_ap, w_sb, b_t, st in [
                        (xk, wk_sb, bk_t, ks),
                        (xq, wq_sb, bq_t, qs),
                    ]:
                        for tb in range(TB):
                            chunks.append((x_ap, w_sb, b_t, st, tb))

                    def mk_qk(npr, x_ap, w_sb, b_t, st, tb):
                        holder = {}

                        def prefetch():
                            holder["slab"] = load_slab(x_ap, tb)

                        def run():
                            qk_chunk(npr, st, w_sb, b_t, tb, holder["slab"])

                        return prefetch, run

                    start_qb = 0 if pr > 0 else 1
                    tasks = [(mk_qk(npr, *ch)) for ch in chunks]
                    nblk = TB - start_qb
                    for i, (pf, run) in enumerate(tasks):
                        blk = start_qb + min(i * nblk // len(tasks), nblk - 1)
                        fillers[blk].append((pf, run))
                else:
                    # pair 3: output projection for finished q-blocks
                    def mk_op(tb, ob):
                        return (None, lambda: oproj_piece(tb, ob))

                    for j in range(TB - 1):      # oproj for qb j in block j+1
                        for tb in range(4 * j, 4 * j + 4):
                            for ob in range(2):
                                fillers[j + 1].append(mk_op(tb, ob))

                for qb in range(TB):
                    # emit prefetches for this block's fillers up front
                    for pf, _ in fillers[qb]:
                        if pf is not None:
                            pf()
                    fq = [run for _, run in fillers[qb]]
                    fi = 0

                    ps_oo = [
                        ops.tile([P, 512], F32, tag=f"ov{i}", name=f"ov{i}")
                        for i in range(2)
                    ]
                    ets = {}
                    first_blk = pr == 0 and qb == 0
                    lag = 4 if first_blk else 2
                    for kt in range(KT):
                        # scores for both heads of the pair (row-tiled pair)
                        ps_s = sps.tile([P, 1024], F32, tag="sc")
                        for i in range(2):
                            nc.tensor.matmul(
                                ps_s[:, ts(i, 512)],
                                ks[ds(64 * i, 64), pr, ts(kt, P)],
                                qs[ds(64 * i, 64), pr, ts(qb, 512)],
                                start=True,
                                stop=True,
                                tile_position=(64 * i, 0),
                            )
                        e_t = ep.tile([P, 1024], F16, tag="exp", name="e_t")
                        nc.scalar.activation(
                            e_t[:],
                            ps_s[:],
                            mybir.ActivationFunctionType.Exp,
                            scale=float(1.0 / (32.0 * WS * WS)),
                        )
                        ets[kt] = e_t

                        if first_blk and kt >= 2:
                            # v projection races ahead of attnv consumption
                            vproj_chunk(kt - 2, vslabs[(kt - 2) // 4])
                        # pair 3's oproj fillers read ao written by
                        # norm_finish, so it must be emitted before them
                        nf_kt = 1 if pr == DC - 1 else 5
                        if kt == nf_kt and pending_norm is not None:
                            norm_finish(*pending_norm)
                            pending_norm = None
                        if (
                            kt % 2 == 1
                            and (pr != DC - 1 or kt >= 3)
                            and fi < len(fq)
                        ):
                            fq[fi]()
                            fi += 1
                        if kt >= lag:
                            k2 = kt - lag
                            for i in range(2):
                                nc.tensor.matmul(
                                    ps_oo[i][0 : D + 1, :],
                                    vp[:, k2, 2 * pr + i, :],
                                    ets[k2][:, ts(i, 512)],
                                    start=(k2 == 0),
                                    stop=False,
                                )
                    if first_blk:
                        vproj_chunk(14, vslabs[3])
                        vproj_chunk(15, vslabs[3])
                    for k2 in range(KT - lag, KT):
                        for i in range(2):
                            nc.tensor.matmul(
                                ps_oo[i][0 : D + 1, :],
                                vp[:, k2, 2 * pr + i, :],
                                ets[k2][:, ts(i, 512)],
                                start=False,
                                stop=(k2 == KT - 1),
                            )
                    while fi < len(fq):
                        fq[fi]()
                        fi += 1
                    sbp, dn = norm_gather(ps_oo)
                    pending_norm = (pr, qb, sbp, dn)

            # tail: last block's normalization + its output projection
            norm_finish(*pending_norm)
            pending_norm = None
            for tb in range(4 * (TB - 1), 4 * TB):
                for ob in range(2):
                    oproj_piece(tb, ob)

    nc.compile()
    return nc


def kernel(q, k, v, padding_mask, sequence_mask, Wq, bq, Wk, bk, Wv, bv, Wo, bo):
    # masks intentionally unused: the reference discards masked_fill results.
    import ml_dtypes

    F8NP = ml_dtypes.float8_e4m3

    if "nc" not in _CACHE:
        _CACHE["nc"] = _build()
    nc = _CACHE["nc"]

    q = np.asarray(q, np.float32)
    k = np.asarray(k, np.float32)
    v = np.asarray(v, np.float32)
    Wq = np.asarray(Wq, np.float32)
    Wk = np.asarray(Wk, np.float32)
    Wv = np.asarray(Wv, np.float32)
    Wo = np.asarray(Wo, np.float32)
    bq = np.asarray(bq, np.float32)
    bk = np.asarray(bk, np.float32)
    bv = np.asarray(bv, np.float32)
    bo = np.asarray(bo, np.float32)

    in_maps = []
    for c in range(8):
        n, g = c // 2, c % 2
        sl = slice(g * GE, (g + 1) * GE)
        # q/k projections run in fp8 with weights pre-scaled by WS; the
        # scale cancels inside the softmax (folded into the exp scale).
        bqk_arr = np.stack(
            [
                (WS * bq[sl]).reshape(DC, P).T,
                (WS * bk[sl]).reshape(DC, P).T,
            ]
        ).astype(np.float32)
        in_maps.append(
            {
                "xq": np.ascontiguousarray(q[n].T.astype(F8NP)),
                "xk": np.ascontiguousarray(k[n].T.astype(F8NP)),
                "xv": np.ascontiguousarray(v[n].T.astype(np.float16)),
                "wq": np.ascontiguousarray((WS * Wq[sl, :].T).astype(F8NP)),
                "wk": np.ascontiguousarray((WS * Wk[sl, :].T).astype(F8NP)),
                "wv": np.ascontiguousarray(Wv[sl, :].T.astype(np.float16)),
                "wo": np.ascontiguousarray(Wo[:, sl].T.astype(np.float16)),
                "bqk": np.ascontiguousarray(bqk_arr),
                "bvr": np.ascontiguousarray(bv[sl][None, :].astype(np.float16)),
            }
        )

    trace = os.environ.get("KERNEL_TRACE") == "1"
    kw = {}
    if trace:
        kw = dict(trace=True, trace_cores=list(range(8)))
    res = run_bass_kernel_spmd(nc, in_maps, core_ids=list(range(8)), **kw)
    if trace:
        _CACHE["exec_time_ns"] = res.exec_time_ns
        _CACHE["mean_exec_time_ns"] = res.mean_exec_time_ns

    outp = np.empty((NB, L, E), np.float32)
    for n in range(NB):
        outp[n] = (
            res.results[2 * n]["out"] + res.results[2 * n + 1]["out"] + bo[None, :]
        )
    return outp


# revision 7
# speedup vs baseline: 1.5598x; 1.0356x over previous
"""Multi-head attention (N=4, L=2048, E=1024, H=16) on 8 Trainium2 cores.

Sharding: core c -> (batch n = c // 2, head-group g = c % 2).  Each core
computes, for its batch and its 8 heads (512 embed dims):
  qp_T/kp_T = (W x^T) in [d, tok] layout (fp8 DoubleRow matmuls; the x64
  fp8 weight scale folds into the softmax exp scale), vp in [tok, d]
  layout (fp16), S_T[k, q] scores with two heads packed in the 128
  partitions via PE row tiling, exp via ACT, attn@v with a ones column
  appended to vp so the softmax denominator accumulates in the same PSUM
  tile, normalization via DVE reciprocal_approx_fast + a col-tiled PE
  replicate matmul + one DVE multiply, then the output projection (fp16).
Host sums the two per-group partial outputs per batch and adds bo.

All DRAM inputs are pre-swizzled on the host into the device layout
([partition, ...] contiguous) so every DMA moves >=4KB contiguous runs
per partition instead of 512B strided descriptors.

Pipeline structure: the softmax exp stream on the ACT engine is the
critical path (~256 x ~1.05us).  All other work (v projection, next
pair's q/k projections, the output projection, normalization) is
interleaved into the attention blocks as PE/DVE filler so ACT never
waits.
"""

import os

import numpy as np

import concourse.bacc as bacc
import concourse.mybir as mybir
import concourse.tile as tile
from concourse.bass import ds, ts
from concourse.bass_utils import run_bass_kernel_spmd

F32 = mybir.dt.float32
F16 = mybir.dt.float16
F8 = mybir.dt.float8e4

E = 1024          # embed
H = 16            # heads (global)
D = 64            # head dim
L = 2048          # sequence length
NB = 4            # batch
GE = 512          # embed dims per head group (8 heads)
P = 128           # partitions
TB = L // 512     # 4 token blocks of 512
EC = E // P       # 8 embed chunks
DC = GE // P      # 4 d-chunks per group == head pairs
KT = L // P       # 16 key-token chunks
LAG = 2           # attnv trails scores by LAG kt-chunks
WS = 64.0         # fp8 weight scale for q/k projections

_CACHE = {}


def _build():
    nc = bacc.Bacc("TRN2", debug=False, enable_asserts=False, num_devices=8)

    # device-layout tensors (host pre-swizzles)
    xq = nc.dram_tensor("xq", [P, TB, EC, 512], F8, kind="ExternalInput").ap()
    xk = nc.dram_tensor("xk", [P, TB, EC, 512], F8, kind="ExternalInput").ap()
    xv = nc.dram_tensor("xv", [P, TB, EC, 512], F16, kind="ExternalInput").ap()
    wq = nc.dram_tensor("wq", [P, EC, GE], F8, kind="ExternalInput").ap()
    wk = nc.dram_tensor("wk", [P, EC, GE], F8, kind="ExternalInput").ap()
    wv = nc.dram_tensor("wv", [P, EC, GE], F16, kind="ExternalInput").ap()
    wo = nc.dram_tensor("wo", [P, DC, E], F16, kind="ExternalInput").ap()
    bqk = nc.dram_tensor("bqk", [2, P, DC], F32, kind="ExternalInput").ap()
    bvr = nc.dram_tensor("bvr", [1, GE], F16, kind="ExternalInput").ap()
    out = nc.dram_tensor("out", [L, E], F32, kind="ExternalOutput").ap()

    with tile.TileContext(nc) as tc, \
         nc.allow_low_precision(reason="fp16/fp8 attention internals by design"):
        with tc.tile_pool(name="persist", bufs=1) as pp, \
             tc.tile_pool(name="wpool", bufs=1) as wp, \
             tc.tile_pool(name="xpool", bufs=8) as xp, \
             tc.tile_pool(name="vxpool", bufs=4) as vxp, \
             tc.tile_pool(name="bias", bufs=1) as bp, \
             tc.tile_pool(name="expp", bufs=6) as ep, \
             tc.tile_pool(name="nrm", bufs=2) as npool, \
             tc.tile_pool(name="otmp", bufs=3) as ot, \
             tc.tile_pool(name="ppsum", bufs=2, space="PSUM") as pps, \
             tc.tile_pool(name="spsum", bufs=2, space="PSUM") as sps, \
             tc.tile_pool(name="opsum", bufs=1, space="PSUM") as ops:
            # persistent SBUF
            vp = pp.tile([P, KT, 8, D + 1], F16)         # vp_aug per head
            ao = pp.tile([P, DC, L], F16)                # normalized attnout_T
            qs = pp.tile([P, DC, L], F16)                # qp_T  [d, pair, tok]
            ks = pp.tile([P, DC, L], F16)                # kp_T
            ones32 = pp.tile([1, P], F32)
            ones = pp.tile([1, P], F16)
            nc.gpsimd.memset(ones32[:], 1.0)
            nc.vector.tensor_copy(ones[:], ones32[:])

            bq_t = bp.tile([P, DC], F32, tag="bq")
            bk_t = bp.tile([P, DC], F32, tag="bk")
            bv_row = bp.tile([1, GE], F16, tag="bv")
            nc.sync.dma_start(bq_t[:], bqk[0])
            nc.sync.dma_start(bk_t[:], bqk[1])
            nc.sync.dma_start(bv_row[:], bvr)

            wq_sb = wp.tile([P, EC, GE], F8, tag="wq")
            wk_sb = wp.tile([P, EC, GE], F8, tag="wk")
            wv_sb = wp.tile([P, EC, GE], F16, tag="wv")
            wo_sb = wp.tile([P, DC, E], F16, tag="wo")
            nc.sync.dma_start(wk_sb[:], wk)
            nc.sync.dma_start(wq_sb[:], wq)
            nc.sync.dma_start(wv_sb[:], wv)
            nc.sync.dma_start(wo_sb[:], wo)

            # ones column of vp_aug (softmax denominator accumulator)
            onescol = bp.tile([P, KT], F32, tag="onescol")
            nc.gpsimd.memset(onescol[:], 1.0)
            nc.vector.tensor_copy(
                vp[:, :, :, D : D + 1],
                onescol[:, :, None, None].to_broadcast([P, KT, 8, 1]),
            )

            # ---------------- emission helpers ----------------
            def load_slab(x_ap, tb):
                x_sb = xp.tile([P, EC, 512], F8, tag="xslab", name="x_sb")
                nc.sync.dma_start(x_sb[:], x_ap[:, tb])
                return x_sb

            def load_vslab(tb):
                x_sb = vxp.tile([P, EC, 512], F16, tag="vslab", name="v_sb")
                nc.sync.dma_start(x_sb[:], xv[:, tb])
                return x_sb

            def qk_chunk(pr, st, w_sb, b_t, tb, slab):
                # fp8 DoubleRow: 2 contraction rows per PE cell
                ps_t = pps.tile([P, 512], F32, tag="pp")
                for e2 in range(EC // 2):
                    nc.tensor.matmul(
                        ps_t[:],
                        w_sb[:, 2 * e2 : 2 * e2 + 2, ts(pr, P)],
                        slab[:, 2 * e2 : 2 * e2 + 2, :],
                        start=(e2 == 0),
                        stop=(e2 == EC // 2 - 1),
                        perf_mode=mybir.MatmulPerfMode.DoubleRow,
                    )
                nc.vector.tensor_scalar_add(
                    st[:, pr, ts(tb, 512)], ps_t[:], b_t[:, pr : pr + 1]
                )

            def vproj_chunk(c, slab):
                ps_t = pps.tile([P, 512], F32, tag="pp")
                for e in range(EC):
                    nc.tensor.matmul(
                        ps_t[:],
                        slab[:, e, ts(c % 4, P)],
                        wv_sb[:, e, :],
                        start=(e == 0),
                        stop=False,
                    )
                nc.tensor.matmul(
                    ps_t[:], ones[:, :P], bv_row[:], start=False, stop=True
                )
                nc.vector.tensor_copy(
                    vp[:, c, :, 0:D],
                    ps_t.rearrange("p (h d) -> p h d", d=D),
                )

            def oproj_piece(tb, ob):
                ps_f = pps.tile([P, 512], F32, tag="pp")
                for dc in range(DC):
                    nc.tensor.matmul(
                        ps_f[:],
                        ao[:, dc, ts(tb, P)],
                        wo_sb[:, dc, ts(ob, 512)],
                        start=(dc == 0),
                        stop=(dc == DC - 1),
                    )
                o_t = ot.tile([P, 512], F32, tag="fout")
                nc.vector.tensor_copy(o_t[:], ps_f[:])
                nc.sync.dma_start(out[ts(tb, P), ts(ob, 512)], o_t[:])

            # normalization part 1 (block end): drain attnv PSUM into SBUF
            def norm_gather(ps_oo):
                sbp = npool.tile([P, 512], F32, tag="sbp", name="sbp")
                dn = npool.tile([1, 2, 512], F32, tag="dn", name="dn")
                nc.vector.tensor_copy(sbp[0:D, :], ps_oo[0][0:D, :])
                nc.vector.tensor_copy(sbp[D:P, :], ps_oo[1][0:D, :])
                nc.vector.tensor_copy(dn[:, 0, :], ps_oo[0][D : D + 1, :])
                nc.vector.tensor_copy(dn[:, 1, :], ps_oo[1][D : D + 1, :])
                return sbp, dn

            # normalization part 2 (mid next block): reciprocal + replicate
            def norm_finish(pr, qb, sbp, dn):
                dninv = npool.tile([1, 2, 512], F32, tag="dninv", name="dninv")
                nc.vector.reciprocal_approx_fast(dninv[:], dn[:])
                dinv = npool.tile([1, 2, 512], F16, tag="dinv", name="dinv")
                nc.vector.tensor_copy(dinv[:], dninv[:])
                ps_r = pps.tile([P, 512], F32, tag="pp", name="ps_r")
                nc.tensor.matmul(
                    ps_r[0:D, :], ones[:, :D], dinv[:, 0, :],
                    start=True, stop=True,
                )
                nc.tensor.matmul(
                    ps_r[D:P, :], ones[:, :D], dinv[:, 1, :],
                    start=True, stop=True, tile_position=(0, 64),
                )
                nc.vector.tensor_tensor(
                    ao[:, pr, ts(qb, 512)],
                    sbp[:],
                    ps_r[:],
                    mybir.AluOpType.mult,
                )

            # ---------------- startup ----------------
            # kproj feeds the scores' stationary operand (all 2048 tokens);
            # qproj tb=0 feeds q-block 0.  The remaining qproj chunks and
            # the whole v projection run inside block (0,0).
            kslabs = [load_slab(xk, tb) for tb in range(TB)]
            qslab0 = load_slab(xq, 0)
            vslabs = {tb: load_vslab(tb) for tb in range(TB)}
            qslabs_rest = {tb: load_slab(xq, tb) for tb in range(1, TB)}
            for tb in range(TB):
                qk_chunk(0, ks, wk_sb, bk_t, tb, kslabs[tb])
            qk_chunk(0, qs, wq_sb, bq_t, 0, qslab0)

            # ---------------- attention with interleaved fillers ----------
            pending_norm = None     # (pr, qb, sbp, dn)

            for pr in range(DC):
                # filler tasks for this pair's 4 blocks: list of callables
                fillers = [[] for _ in range(TB)]
                if pr < DC - 1:
                    # next pair's q/k projection: 8 chunks
                    npr = pr + 1
                    chunks = []
                    for x_ap, w_sb, b_t, st in [
                        (xk, wk_sb, bk_t, ks),
                        (xq, wq_sb, bq_t, qs),
                    ]:
                        for tb in range(TB):
                            chunks.append((x_ap, w_sb, b_t, st, tb))

                    def mk_qk(npr, x_ap, w_sb, b_t, st, tb):
                        holder = {}

                        def prefetch():
                            holder["slab"] = load_slab(x_ap, tb)

                        def run():
                            qk_chunk(npr, st, w_sb, b_t, tb, holder["slab"])

                        return prefetch, run

                    start_qb = 0 if pr > 0 else 1
                    tasks = [(mk_qk(npr, *ch)) for ch in chunks]
                    nblk = TB - start_qb
                    for i, (pf, run) in enumerate(tasks):
                        blk = start_qb + min(i * nblk // len(tasks), nblk - 1)
                        fillers[blk].append((pf, run))
                else:
                    # pair 3: output projection for finished q-blocks
                    def mk_op(tb, ob):
                        return (None, lambda: oproj_piece(tb, ob))

                    for j in range(TB - 1):      # oproj for qb j in block j+1
                        for tb in range(4 * j, 4 * j + 4):
                            for ob in range(2):
                                fillers[j + 1].append(mk_op(tb, ob))

                for qb in range(TB):
                    # emit prefetches for this block's fillers up front
                    for pf, _ in fillers[qb]:
                        if pf is not None:
                            pf()
                    fq = [run for _, run in fillers[qb]]
                    fi = 0

                    ps_oo = [
                        ops.tile([P, 512], F32, tag=f"ov{i}", name=f"ov{i}")
                        for i in range(2)
                    ]
                    ets = {}
                    first_blk = pr == 0 and qb == 0
                    lag = 4 if first_blk else LAG
                    for kt in range(KT):
                        # scores for both heads of the pair (row-tiled pair)
                        ps_s = sps.tile([P, 1024], F32, tag="sc")
                        for i in range(2):
                            nc.tensor.matmul(
                                ps_s[:, ts(i, 512)],
                                ks[ds(64 * i, 64), pr, ts(kt, P)],
                                qs[ds(64 * i, 64), pr, ts(qb, 512)],
                                start=True,
                                stop=True,
                                tile_position=(64 * i, 0),
                            )
                        e_t = ep.tile([P, 1024], F16, tag="exp", name="e_t")
                        nc.scalar.activation(
                            e_t[:],
                            ps_s[:],
                            mybir.ActivationFunctionType.Exp,
                            scale=float(1.0 / (32.0 * WS * WS)),
                        )
                        ets[kt] = e_t

                        if first_blk:
                            # v projection + remaining qproj race ahead of
                            # the attnv consumption (lag 4)
                            vproj_chunk(kt, vslabs[kt // 4])
                            if kt in (3, 7, 11):
                                tbq = (kt + 1) // 4
                                qk_chunk(
                                    0, qs, wq_sb, bq_t, tbq, qslabs_rest[tbq]
                                )
                        # pair 3's oproj fillers read ao written by
                        # norm_finish, so it must be emitted before them
                        nf_kt = 1 if pr == DC - 1 else 5
                        if kt == nf_kt and pending_norm is not None:
                            norm_finish(*pending_norm)
                            pending_norm = None
                        if (
                            kt % 2 == 1
                            and (pr != DC - 1 or kt >= 3)
                            and fi < len(fq)
                        ):
                            fq[fi]()
                            fi += 1
                        if kt >= lag:
                            k2 = kt - lag
                            for i in range(2):
                                nc.tensor.matmul(
                                    ps_oo[i][0 : D + 1, :],
                                    vp[:, k2, 2 * pr + i, :],
                                    ets[k2][:, ts(i, 512)],
                                    start=(k2 == 0),
                                    stop=False,
                                )
                    for k2 in range(KT - lag, KT):
                        for i in range(2):
                            nc.tensor.matmul(
                                ps_oo[i][0 : D + 1, :],
                                vp[:, k2, 2 * pr + i, :],
                                ets[k2][:, ts(i, 512)],
                                start=False,
                                stop=(k2 == KT - 1),
                            )
                    while fi < len(fq):
                        fq[fi]()
                        fi += 1
                    sbp, dn = norm_gather(ps_oo)
                    pending_norm = (pr, qb, sbp, dn)

            # tail: last block's normalization + its output projection
            norm_finish(*pending_norm)
            pending_norm = None
            for tb in range(4 * (TB - 1), 4 * TB):
                for ob in range(2):
                    oproj_piece(tb, ob)

    nc.compile()
    return nc


def _swizzle_x(xt, dtype):
    # [E, L] -> [P, TB, EC, 512]  with E = eo*P + p, L = tb*512 + t
    arr = xt.reshape(EC, P, TB, 512).transpose(1, 2, 0, 3)
    return np.ascontiguousarray(arr.astype(dtype))


def _swizzle_w(wt, dtype, inner):
    # [E_in, F] -> [P, E_in//P, F]
    arr = wt.reshape(inner, P, wt.shape[1]).transpose(1, 0, 2)
    return np.ascontiguousarray(arr.astype(dtype))


def kernel(q, k, v, padding_mask, sequence_mask, Wq, bq, Wk, bk, Wv, bv, Wo, bo):
    # masks intentionally unused: the reference discards masked_fill results.
    import ml_dtypes

    F8NP = ml_dtypes.float8_e4m3

    if "nc" not in _CACHE:
        _CACHE["nc"] = _build()
    nc = _CACHE["nc"]

    q = np.asarray(q, np.float32)
    k = np.asarray(k, np.float32)
    v = np.asarray(v, np.float32)
    Wq = np.asarray(Wq, np.float32)
    Wk = np.asarray(Wk, np.float32)
    Wv = np.asarray(Wv, np.float32)
    Wo = np.asarray(Wo, np.float32)
    bq = np.asarray(bq, np.float32)
    bk = np.asarray(bk, np.float32)
    bv = np.asarray(bv, np.float32)
    bo = np.asarray(bo, np.float32)

    in_maps = []
    for c in range(8):
        n, g = c // 2, c % 2
        sl = slice(g * GE, (g + 1) * GE)
        # q/k projections run in fp8 with weights pre-scaled by WS; the
        # scale cancels inside the softmax (folded into the exp scale).
        bqk_arr = np.stack(
            [
                (WS * bq[sl]).reshape(DC, P).T,
                (WS * bk[sl]).reshape(DC, P).T,
            ]
        ).astype(np.float32)
        in_maps.append(
            {
                "xq": _swizzle_x(q[n].T, F8NP),
                "xk": _swizzle_x(k[n].T, F8NP),
                "xv": _swizzle_x(v[n].T, np.float16),
                "wq": _swizzle_w(WS * Wq[sl, :].T, F8NP, EC),
                "wk": _swizzle_w(WS * Wk[sl, :].T, F8NP, EC),
                "wv": _swizzle_w(Wv[sl, :].T, np.float16, EC),
                "wo": _swizzle_w(Wo[:, sl].T, np.float16, DC),
                "bqk": np.ascontiguousarray(bqk_arr),
                "bvr": np.ascontiguousarray(bv[sl][None, :].astype(np.float16)),
            }
        )

    trace = os.environ.get("KERNEL_TRACE") == "1"
    kw = {}
    if trace:
        kw = dict(trace=True, trace_cores=list(range(8)))
    res = run_bass_kernel_spmd(nc, in_maps, core_ids=list(range(8)), **kw)
    if trace:
        _CACHE["exec_time_ns"] = res.exec_time_ns
        _CACHE["mean_exec_time_ns"] = res.mean_exec_time_ns

    outp = np.empty((NB, L, E), np.float32)
    for n in range(NB):
        outp[n] = (
            res.results[2 * n]["out"] + res.results[2 * n + 1]["out"] + bo[None, :]
        )
    return outp
